# revision 22
# baseline (speedup 1.0000x reference)
"""Trainium2 Bass kernel for CausalSelfAttention KV-cache decode step.

Problem: B=16, T_new=8, C=1024, H=16, hd=64, T_past=4096.
  qkv = x @ W_attn.T + b_attn ; split q,k,v ; k/v appended to cache;
  att = softmax(q k^T / 8) causal over last 8 rows ; y = att v ;
  out = y @ W_proj.T + b_proj.  Returns (out, k_full, v_full).

Sharding: tensor-parallel over heads — 2 heads per NeuronCore x 8 cores.
Each core streams its 2 heads' KV cache (the memory-bound part), computes
attention + its slice of the output projection; host sums the partial
projections (cheap: 8 x 0.5MB) and concatenates the KV cache on host
(pure data movement, no compute).

Device-side structure (per core, per (batch, head-pair) unit):
  - scores^T: 32 matmuls, stationary = K^T chunk (128x128 fp16, FWL),
    moving = block-diag q^T (128x16). S^T comes out with keys on
    partitions so exp runs full-width and feeds att@v directly.
  - att@v: stationary = V chunk (128x128 fp16, FWL), moving = exp(S^T)
    slice (128x16), accumulating y^T (128x16) in PSUM. The softmax
    normalizer l is accumulated in the same PSUM bank by two ones-vector
    matmuls over exp(S^T) (single has_written accumulation group).
  - normalizer broadcast via a Kc=1 ones matmul; normalization fused
    into the PSUM->SBUF move of y^T.
Units are software-pipelined (scores of unit b+1 are issued before
att@v of unit b) so the PE never stalls on the ACT exp latency.
"""

import numpy as np

B, T, C, H, TP = 16, 8, 1024, 16, 4096
HD = C // H            # 64
NCORES = 8
HPC = H // NCORES      # 2 heads per core
NTOK = B * T           # 128
NCH = TP // 128        # 32 score chunks of 128 keys
TFULL = TP + T

F16 = np.float16
F32 = np.float32

_BUILT = {}


def _build_bass():
    import concourse.mybir as mybir
    import concourse.tile as tile
    from concourse import bacc
    from concourse.masks import make_identity

    f32 = mybir.dt.float32
    f16 = mybir.dt.float16
    Exp = mybir.ActivationFunctionType.Exp

    nc = bacc.Bacc(None, target_bir_lowering=False)

    # ---- DRAM I/O (per-core values supplied by host) ----
    xT_d = nc.dram_tensor("xT", [128, C], f32, kind="ExternalInput")
    wT_d = nc.dram_tensor("wT", [128, 3 * C], f32, kind="ExternalInput")
    bs_d = nc.dram_tensor("bs", [128, 3], f32, kind="ExternalInput")
    wp_d = nc.dram_tensor("wp", [128, C], f32, kind="ExternalInput")
    msk_d = nc.dram_tensor("msk", [8, 16], f16, kind="ExternalInput")
    kT_d = nc.dram_tensor("kT", [B, 128, TP], f16, kind="ExternalInput")
    v2_d = nc.dram_tensor("v2", [B, 128, TP], f16, kind="ExternalInput")

    outp_d = nc.dram_tensor("outp", [NTOK, C], f32, kind="ExternalOutput")
    knT_d = nc.dram_tensor("knT", [128, NTOK], f32, kind="ExternalOutput")
    vnT_d = nc.dram_tensor("vnT", [128, NTOK], f32, kind="ExternalOutput")

    with tile.TileContext(nc) as tc:
        with (
            tc.tile_pool(name="const", bufs=1) as constp,
            tc.tile_pool(name="kv", bufs=5) as kvp,
            tc.tile_pool(name="work", bufs=3) as workp,
            tc.tile_pool(name="ps_st", bufs=2, space="PSUM") as ps_st,
            tc.tile_pool(name="ps_sn", bufs=2, space="PSUM") as ps_sn,
            tc.tile_pool(name="ps_y", bufs=2, space="PSUM") as ps_y,
            tc.tile_pool(name="ps_misc", bufs=2, space="PSUM") as ps_misc,
        ):
            # ---- constants ----
            ident = constp.tile([128, 128], f32)
            make_identity(nc, ident[:])
            # all non-KV loads go on the scalar HWDGE ring so the sync ring
            # starts streaming the KV cache immediately
            xT_s = constp.tile([128, C], f32)
            nc.scalar.dma_start(xT_s[:], xT_d[:])
            wT_s = constp.tile([128, 3 * C], f32)
            nc.sync.dma_start(wT_s[:], wT_d[:])
            bs_s = constp.tile([128, 3], f32)
            nc.scalar.dma_start(bs_s[:], bs_d[:])
            msk_s = constp.tile([8, 16], f16)
            nc.scalar.dma_start(msk_s[:], msk_d[:])
            wp_s = constp.tile([128, C], f32)
            ones_c = constp.tile([128, 1], f16)
            nc.vector.memset(ones_c[:], 1.0)
            ones_r = constp.tile([1, 128], f32)
            nc.vector.memset(ones_r[:], 1.0)

            # ---- phase 0: qkv^T = W_slice @ x^T + b  (all 128 tokens at once) ----
            qkvT = constp.tile([128, 384], f32)
            for m in range(3):
                psm = ps_misc.tile([128, 128], f32, tag="misc", name=f"psm{m}")
                for k in range(8):
                    nc.tensor.matmul(
                        psm[:],
                        wT_s[:, (k * 3 + m) * 128:(k * 3 + m + 1) * 128],
                        xT_s[:, k * 128:(k + 1) * 128],
                        start=(k == 0),
                        stop=(k == 7),
                    )
                nc.vector.tensor_scalar_add(
                    qkvT[:, m * 128:(m + 1) * 128], psm[:], bs_s[:, m:m + 1]
                )

            # fp16 cast of new-K^T (for last score chunk lhsT)
            kn16 = constp.tile([128, NTOK], f16)
            nc.vector.tensor_copy(kn16[:], qkvT[:, 128:256])

            # all block-diagonal q^T tiles at once: qbd_all[:, b*16:(b+1)*16]
            qbd_all = constp.tile([128, 16 * B], f16)
            nc.vector.memset(qbd_all[:], 0.0)
            nc.vector.tensor_copy(
                qbd_all[0:64, :].rearrange("p (u s) -> p u s", s=16)[:, :, 0:8],
                qkvT[0:64, 0:128].rearrange("p (u s) -> p u s", s=8),
            )
            nc.vector.tensor_copy(
                qbd_all[64:128, :].rearrange("p (u s) -> p u s", s=16)[:, :, 8:16],
                qkvT[64:128, 0:128].rearrange("p (u s) -> p u s", s=8),
            )

            yT_all = constp.tile([128, NTOK], f32)

            # ---- software-pipelined units ----
            state = {}

            def front(b):
                kts = kvp.tile([128, TP], f16, tag="kt", name=f"kts{b}")
                nc.sync.dma_start(kts[:], kT_d[b])
                v2s = kvp.tile([128, TP], f16, tag="v2", name=f"v2s{b}")
                nc.scalar.dma_start(v2s[:], v2_d[b])
                if b == 4:
                    # W_proj load slotted mid-stream; lands well before the
                    # first projection half needs it
                    nc.scalar.dma_start(wp_s[:], wp_d[:])

                qbd = qbd_all[:, b * 16:(b + 1) * 16]

                stA = ps_st.tile([128, 512], f32, tag="stA", name=f"stA{b}")
                for idx in range(NCH):
                    nc.tensor.matmul(
                        stA[:, idx * 16:(idx + 1) * 16],
                        kts[:, idx * 128:(idx + 1) * 128],
                        qbd,
                        start=True,
                        stop=True,
                    )
                # sn bank regions: scores_new [0:8, 0:16]; l [0:1, 16:272];
                # l_new [0:1, 272:288]
                sn = ps_sn.tile([128, 512], f32, tag="sn", name=f"sn{b}")
                nc.tensor.matmul(
                    sn[0:8, 0:16], kn16[:, b * 8:b * 8 + 8], qbd,
                    start=True, stop=True,
                )

                expS = workp.tile([128, 512], f16, tag="expS", name=f"expS{b}")
                nc.scalar.activation(expS[:], stA[:], Exp, scale=0.125)
                expN = workp.tile([8, 16], f16, tag="expN", name=f"expN{b}")
                nc.scalar.activation(expN[:], sn[0:8, 0:16], Exp, scale=0.125)
                nc.vector.tensor_mul(expN[:], expN[:], msk_s[:])

                # v_new natural (token, c_loc) for this batch
                vtp = ps_misc.tile([8, 128], f32, tag="misc", name=f"vtp{b}")
                nc.tensor.transpose(
                    vtp[:], qkvT[:, 256 + b * 8:256 + b * 8 + 8], ident[:]
                )
                vne = workp.tile([8, 128], f16, tag="vne", name=f"vne{b}")
                nc.vector.tensor_copy(vne[:], vtp[:])

                # softmax denominator, early so the inverse is ready
                # before back(b): l = ones^T exp(S) accumulated in sn
                nc.tensor.matmul(
                    sn[0:1, 16:272], ones_c[:], expS[:, 0:256],
                    start=True, stop=False,
                )
                nc.tensor.matmul(
                    sn[0:1, 16:272], ones_c[:], expS[:, 256:512],
                    start=False, stop=False,
                )
                nc.tensor.matmul(
                    sn[0:1, 272:288], ones_c[0:8, :], expN[:],
                    start=False, stop=True,
                )
                invl = workp.tile([1, 16], f32, tag="invl", name=f"invl{b}")
                nc.vector.tensor_reduce(
                    invl[:],
                    sn[0:1, 16:272].rearrange("p (i q) -> p q i", q=16),
                    axis=mybir.AxisListType.X,
                    op=mybir.AluOpType.add,
                )
                nc.vector.tensor_add(invl[:], invl[:], sn[0:1, 272:288])
                nc.vector.reciprocal(invl[:], invl[:])

                state[b] = (v2s, expS, expN, vne, invl)

            def back(b):
                v2s, expS, expN, vne, invl = state.pop(b)

                # broadcast 1/l across partitions via Kc=1 matmul
                bcp = ps_misc.tile([128, 16], f32, tag="misc", name=f"bcp{b}")
                nc.tensor.matmul(bcp[:], ones_r[:], invl[:], start=True, stop=True)
                invb = workp.tile([128, 16], f32, tag="invb", name=f"invb{b}")
                nc.vector.tensor_copy(invb[:], bcp[:])

                yp = ps_y.tile([128, 16], f32, tag="yp", name=f"yp{b}")
                for idx in range(NCH):
                    nc.tensor.matmul(
                        yp[:],
                        v2s[:, idx * 128:(idx + 1) * 128],
                        expS[:, idx * 16:(idx + 1) * 16],
                        start=(idx == 0), stop=False,
                    )
                nc.tensor.matmul(yp[:], vne[:], expN[:], start=False, stop=True)

                # normalized y^T written straight into yT_all[c_loc, token]
                nc.vector.tensor_mul(
                    yT_all[0:64, b * 8:b * 8 + 8], yp[0:64, 0:8], invb[0:64, 0:8]
                )
                nc.vector.tensor_mul(
                    yT_all[64:128, b * 8:b * 8 + 8], yp[64:128, 8:16],
                    invb[64:128, 8:16]
                )

            # ---- output projection partial: out_p = y_loc @ Wp_slice^T ----
            # computed in token halves so most of it overlaps the stream
            outS = constp.tile([NTOK, C], f32)

            def proj_half(h):
                for n in range(2):
                    pp = ps_misc.tile(
                        [64, 512], f32, tag="misc", name=f"pp{h}{n}"
                    )
                    nc.tensor.matmul(
                        pp[:],
                        yT_all[:, h * 64:(h + 1) * 64],
                        wp_s[:, n * 512:(n + 1) * 512],
                        start=True, stop=True,
                    )
                    nc.vector.tensor_copy(
                        outS[h * 64:(h + 1) * 64, n * 512:(n + 1) * 512], pp[:]
                    )

            for b in range(B):
                front(b)
                if b > 0:
                    back(b - 1)
                if b == 9:
                    proj_half(0)
            back(B - 1)
            proj_half(1)

            # outputs issued last so they never block the KV streaming rings
            nc.scalar.dma_start(outp_d[:], outS[:])
            nc.scalar.dma_start(knT_d[:], qkvT[:, 128:256])
            nc.scalar.dma_start(vnT_d[:], qkvT[:, 256:384])

    nc.finalize()
    return nc


def _prep_inputs(x, past_k, past_v, W_attn, b_attn, W_proj, b_proj):
    """Host-side packing of per-core device inputs."""
    x = np.asarray(x, F32)
    past_k = np.asarray(past_k, F32)
    past_v = np.asarray(past_v, F32)
    W_attn = np.asarray(W_attn, F32)
    b_attn = np.asarray(b_attn, F32)
    W_proj = np.asarray(W_proj, F32)

    # x^T packed: [p, k*128 + tok] = x[tok, 128k + p]
    xT = np.ascontiguousarray(
        x.reshape(NTOK, C).T.reshape(8, 128, NTOK).transpose(1, 0, 2).reshape(128, C)
    )

    # K^T fp16 packed: (core, b, (h,d), (j, r, pp)) where t = 512j + 4pp + r
    ktf = past_k.astype(F16).reshape(B, NCORES, HPC, 8, 128, 4, HD)
    ktf = ktf.transpose(1, 0, 2, 6, 3, 5, 4).reshape(NCORES, B, 128, TP)
    ktf = np.ascontiguousarray(ktf)

    # V fp16 packed: (core, b, p, (j, r), (h, d)) with t = 512j + 4p + r
    vf = past_v.astype(F16).reshape(B, NCORES, HPC, 8, 128, 4, HD)
    vf = vf.transpose(1, 0, 4, 3, 5, 2, 6).reshape(NCORES, B, 128, TP)
    vf = np.ascontiguousarray(vf)

    # causal mask for the 8 new keys, duplicated for both heads
    tri = (np.arange(8)[:, None] <= np.arange(8)[None, :]).astype(F16)
    msk = np.concatenate([tri, tri], axis=1)

    in_maps = []
    for c in range(NCORES):
        h0 = HPC * c
        rows = np.r_[h0 * HD:(h0 + HPC) * HD,
                     C + h0 * HD:C + (h0 + HPC) * HD,
                     2 * C + h0 * HD:2 * C + (h0 + HPC) * HD]
        Wsl = W_attn[rows]                      # (384, C)
        wT = Wsl.T.reshape(8, 128, 3, 128).transpose(1, 0, 2, 3).reshape(128, 3 * C)
        bs = b_attn[rows].reshape(3, 128).T     # (128, 3)
        wp = W_proj.T[h0 * HD:(h0 + HPC) * HD, :]  # (128, C)
        in_maps.append({
            "xT": xT,
            "wT": np.ascontiguousarray(wT),
            "bs": np.ascontiguousarray(bs),
            "wp": np.ascontiguousarray(wp),
            "msk": msk,
            "kT": ktf[c],
            "v2": vf[c],
        })
    return in_maps


LAST_RESULT = None


def kernel(x, past_k, past_v, W_attn, b_attn, W_proj, b_proj):
    global LAST_RESULT
    from concourse.bass_utils import run_bass_kernel_spmd

    if "nc" not in _BUILT:
        _BUILT["nc"] = _build_bass()
    nc = _BUILT["nc"]

    in_maps = _prep_inputs(x, past_k, past_v, W_attn, b_attn, W_proj, b_proj)

    import os
    trace = bool(int(os.environ.get("KERNEL_TRACE", "0")))
    kw = {}
    if int(os.environ.get("KERNEL_TRACE_ALL", "0")):
        kw["trace_cores"] = list(range(NCORES))
    res = run_bass_kernel_spmd(
        nc, in_maps, core_ids=list(range(NCORES)), trace=trace, **kw
    )
    LAST_RESULT = res

    # ---- host-side unshard ----
    out = np.zeros((NTOK, C), F32)
    k_new = np.empty((B, H, T, HD), F32)
    v_new = np.empty((B, H, T, HD), F32)
    for c in range(NCORES):
        r = res.results[c]
        out += r["outp"]
        kn = r["knT"].reshape(HPC, HD, B, T).transpose(2, 0, 3, 1)
        vn = r["vnT"].reshape(HPC, HD, B, T).transpose(2, 0, 3, 1)
        k_new[:, HPC * c:HPC * (c + 1)] = kn
        v_new[:, HPC * c:HPC * (c + 1)] = vn

    out = out.reshape(B, T, C) + np.asarray(b_proj, F32)
    k_full = np.concatenate([np.asarray(past_k, F32), k_new], axis=2)
    v_full = np.concatenate([np.asarray(past_v, F32), v_new], axis=2)
    return out, k_full, v_full
